# revision 8
# baseline (speedup 1.0000x reference)
"""GATv2 layer (2 heads) on 8 Trainium2 NeuronCores.

Strategy (sharding_hint): nodes sharded across 8 cores (6250 each); edges
partitioned by destination-node owner and sorted by dst so segment softmax
and segment-sum stay core-local. Each core computes the full x_l table
(replicated phase-0) plus its local x_r shard; per-edge features are
fetched with SWDGE dma_gather (int16 indices -> the 50k-row x_l table is
addressed as a lo [0,32768) / hi [32768,50048) pair of base offsets).

Per-edge pipeline ([edge, channel] layout, fp16):
  s    = xl[src] + xr[dst]                       (DVE)
  lr   = max(s, 0.2*s)                           (ACT scale-copy + DVE max)
  alpha= sum_c att_c * lr_c  per head            (DVE mult + reduce)
  w    = exp(alpha)  (fp32, unshifted softmax)   (ACT)
  msg  = xl[src] * w   per head                  (DVE, step-0 broadcast AP)
  agg  : psum[nodes,130] += S^T @ [msg | w]      (PE; S = one-hot(dst) via
                                                  DVE is_equal vs iota)
Per node tile (128 nodes): num/den divide, PE transpose, final linear
(+folded bias), LayerNorm, DMA out.
"""
import sys

sys.path.insert(0, "/opt/trn_rl_repo")

import numpy as np

import concourse.bass as bass
import concourse.tile as tile
from concourse import bacc, mybir
from concourse.bass_utils import run_bass_kernel_spmd

# ---- problem constants (hardcoded per contract) ----
N = 50000
N_EDGES = 800000
D_IN = 128
D_OUT = 64
H = 2
CH = H * D_OUT          # 128
NEG_SLOPE = 0.2
EPS = 1e-5

NCORES = 8
NPC = N // NCORES       # 6250 nodes per core
TILE = 128
NT = (NPC + TILE - 1) // TILE   # 49 node tiles per core
NPAD = NT * TILE                # 6272
EB = 128                        # edges per block
CHUNK = 16                      # blocks per fused DVE chunk
SPAN_MAX_TILES = 3
SPAN_MAX_BLOCKS = 48
LO_LIM = 32768                  # int16 index limit
XL_ROWS = 50048                 # padded full table rows
XR_ROWS = NPAD + TILE           # local table + zero rows at [NPAD, NPAD+128)
NQ = 4                          # SWDGE queues

f32 = mybir.dt.float32
f16 = mybir.dt.float16
i16 = mybir.dt.int16

_CACHE = {}


def _wrap_idx(flat):
    """int16 gather index layout: edge i -> partition i%16 col i//16,
    replicated across the 8 gpsimd core groups (16 partitions each)."""
    L = len(flat)
    assert L % 16 == 0
    w = np.zeros((128, L // 16), np.int16)
    cols = np.asarray(flat, np.int64).reshape(-1, 16).T.astype(np.int16)  # [16, L/16]
    for rep in range(8):
        w[16 * rep:16 * rep + 16, :] = cols
    return w


def _preprocess(E):
    """Sort edges by dst, shard by dst owner, build the per-core block
    schedule (uniform across cores for SPMD) and gather-index arrays."""
    src = np.concatenate([E[0].astype(np.int64), np.arange(N, dtype=np.int64)])
    dst = np.concatenate([E[1].astype(np.int64), np.arange(N, dtype=np.int64)])
    order = np.argsort(dst, kind="stable")
    src = src[order]
    dst = dst[order]

    # per-(core, tile, half) edge lists
    edge_lists = [[None] * NT for _ in range(NCORES)]
    for c in range(NCORES):
        base = c * NPC
        for t in range(NT):
            n0 = base + t * TILE
            n1 = min(base + (t + 1) * TILE, base + NPC)
            e0 = np.searchsorted(dst, n0, "left")
            e1 = np.searchsorted(dst, n1, "left")
            s_t = src[e0:e1]
            d_t = dst[e0:e1] - n0          # 0..127 within tile
            lo_m = s_t < LO_LIM
            edge_lists[c][t] = (s_t[lo_m], d_t[lo_m], s_t[~lo_m], d_t[~lo_m])

    # uniform block counts
    B_lo = np.zeros(NT, np.int64)
    B_hi = np.zeros(NT, np.int64)
    for t in range(NT):
        for c in range(NCORES):
            lo, _, hi, _ = edge_lists[c][t]
            B_lo[t] = max(B_lo[t], -(-len(lo) // EB))
            B_hi[t] = max(B_hi[t], -(-len(hi) // EB))
        if B_lo[t] + B_hi[t] == 0:
            B_lo[t] = 1  # shouldn't happen (self loops), keep psum valid

    # spans: consecutive tiles, capped
    spans = []  # (t0, t1, b0, Blo_span, Bhi_span)
    t = 0
    b0 = 0
    while t < NT:
        t0 = t
        blk = 0
        while (t < NT and t - t0 < SPAN_MAX_TILES
               and blk + B_lo[t] + B_hi[t] <= SPAN_MAX_BLOCKS):
            blk += B_lo[t] + B_hi[t]
            t += 1
        assert t > t0
        spans.append((t0, t, b0, int(B_lo[t0:t].sum()), int(B_hi[t0:t].sum())))
        b0 += blk
    NBLK = b0

    # block -> (tile, is_last_of_tile); per span order: lo blocks (tiles asc)
    # then hi blocks (tiles asc)
    blk_tile = np.zeros(NBLK, np.int64)
    for (t0, t1, sb0, blo, bhi) in spans:
        b = sb0
        for t in range(t0, t1):
            blk_tile[b:b + B_lo[t]] = t
            b += B_lo[t]
        for t in range(t0, t1):
            blk_tile[b:b + B_hi[t]] = t
            b += B_hi[t]
    last_blk_of_tile = np.zeros(NBLK, bool)
    for t in range(NT):
        idxs = np.nonzero(blk_tile == t)[0]
        last_blk_of_tile[idxs[-1]] = True
    first_seen = set()
    first_blk_of_tile = np.zeros(NBLK, bool)
    for b in range(NBLK):
        t = int(blk_tile[b])
        if t not in first_seen:
            first_seen.add(t)
            first_blk_of_tile[b] = True

    # per-core edge streams in block order
    per_core = []
    for c in range(NCORES):
        xl_flat = np.zeros(NBLK * EB, np.int64)
        xr_flat = np.full(NBLK * EB, NPAD, np.int64)   # pad -> zero row
        dst_rel = np.full(NBLK * EB, -1.0, np.float32)
        base = c * NPC
        for (t0, t1, sb0, blo, bhi) in spans:
            b = sb0
            for half in (0, 1):
                for t in range(t0, t1):
                    lo_s, lo_d, hi_s, hi_d = edge_lists[c][t]
                    s_e, d_e = (lo_s, lo_d) if half == 0 else (hi_s, hi_d)
                    nb = B_lo[t] if half == 0 else B_hi[t]
                    off = b * EB
                    ne = len(s_e)
                    if half == 0:
                        xl_flat[off:off + ne] = s_e          # < 32768
                    else:
                        xl_flat[off:off + ne] = s_e - LO_LIM
                    xr_flat[off:off + ne] = (d_e + t * TILE)  # local node id
                    dst_rel[off:off + ne] = d_e
                    b += nb
        # wrap per gather call: per span, lo-call then hi-call for xl;
        # single call for xr. Wrapping is per-16 within the call, but the
        # wrap of a concatenation of whole blocks equals concatenation of
        # wraps of whole blocks (each block is 128 = 8*16 edges), so one
        # global wrap with per-call column slicing is correct.
        xl_w = _wrap_idx(xl_flat)
        xr_w = _wrap_idx(xr_flat)
        # dst16 [128 lanes, NBLK]: edge (blk, lane) value
        dst16 = dst_rel.reshape(NBLK, EB).T.astype(np.float16).copy()
        per_core.append((xl_w, xr_w, dst16))

    sched = dict(spans=spans, B_lo=B_lo, B_hi=B_hi, NBLK=NBLK,
                 blk_tile=blk_tile, last_blk=last_blk_of_tile,
                 first_blk=first_blk_of_tile)
    return sched, per_core


def _bcast(ap, insert_at, dims):
    """Insert step-0 dims into an AP (broadcast)."""
    new = list(ap.ap)
    for d in reversed(dims):
        new.insert(insert_at, [0, d])
    return bass.AP(tensor=ap.tensor, offset=ap.offset, ap=new)


def _build(sched, ln_trivial):
    spans = sched["spans"]
    B_lo, B_hi = sched["B_lo"], sched["B_hi"]
    NBLK = sched["NBLK"]
    blk_tile = sched["blk_tile"]
    last_blk = sched["last_blk"]
    first_blk = sched["first_blk"]
    NFT = (XL_ROWS // TILE)      # 391 full-table tiles

    nc = bacc.Bacc("TRN2", target_bir_lowering=False, debug=False,
                   num_swdge_queues=NQ)
    P = 128

    X_full = nc.declare_dram_parameter("X_full", [XL_ROWS, D_IN], f32, isOutput=False)
    X_local = nc.declare_dram_parameter("X_local", [NPAD, D_IN], f32, isOutput=False)
    xl_idx = nc.declare_dram_parameter("xl_idx", [P, NBLK * 8], i16, isOutput=False)
    xr_idx = nc.declare_dram_parameter("xr_idx", [P, NBLK * 8], i16, isOutput=False)
    dst16_p = nc.declare_dram_parameter("dst16", [P, NBLK], f16, isOutput=False)
    W_l_p = nc.declare_dram_parameter("W_l", [D_IN, CH], f32, isOutput=False)
    W_r_p = nc.declare_dram_parameter("W_r", [D_IN, CH], f32, isOutput=False)
    b_l_p = nc.declare_dram_parameter("b_l", [1, CH], f32, isOutput=False)
    b_r_p = nc.declare_dram_parameter("b_r", [1, CH], f32, isOutput=False)
    att_p = nc.declare_dram_parameter("att_t", [P, CH], f16, isOutput=False)
    Wf_p = nc.declare_dram_parameter("W_f16", [CH, D_OUT], f16, isOutput=False)
    bf_p = nc.declare_dram_parameter("bf_eff", [1, D_OUT], f16, isOutput=False)
    id32_p = nc.declare_dram_parameter("id_f32", [P, P], f32, isOutput=False)
    id16_p = nc.declare_dram_parameter("id_f16", [P, P], f16, isOutput=False)
    iota_p = nc.declare_dram_parameter("iota_t", [P, P], f16, isOutput=False)
    gam_p = nc.declare_dram_parameter("gamma_t", [P, D_OUT], f32, isOutput=False)
    bet_p = nc.declare_dram_parameter("beta_t", [P, D_OUT], f32, isOutput=False)

    h_out = nc.declare_dram_parameter("h_out", [NPAD, D_OUT], f32, isOutput=True)

    xl16 = nc.dram_tensor("xl16", [XL_ROWS, CH], f16)
    xr16 = nc.dram_tensor("xr16", [XR_ROWS, CH], f16)

    AT = mybir.AluOpType
    AF = mybir.ActivationFunctionType

    with tile.TileContext(nc) as tc:
        import contextlib
        with contextlib.ExitStack() as ctx:
            consts = ctx.enter_context(tc.tile_pool(name="consts", bufs=1))
            # ---- load constants ----
            w_l = consts.tile([D_IN, CH], f32)
            nc.sync.dma_start(out=w_l[:], in_=W_l_p[:])
            w_r = consts.tile([D_IN, CH], f32)
            nc.sync.dma_start(out=w_r[:], in_=W_r_p[:])
            b_l = consts.tile([1, CH], f32)
            nc.sync.dma_start(out=b_l[:], in_=b_l_p[:])
            b_r = consts.tile([1, CH], f32)
            nc.sync.dma_start(out=b_r[:], in_=b_r_p[:])
            att_t = consts.tile([P, CH], f16)
            nc.sync.dma_start(out=att_t[:], in_=att_p[:])
            w_f = consts.tile([CH, D_OUT], f16)
            nc.sync.dma_start(out=w_f[:], in_=Wf_p[:])
            bf_e = consts.tile([1, D_OUT], f16)
            nc.sync.dma_start(out=bf_e[:], in_=bf_p[:])
            id32 = consts.tile([P, P], f32)
            nc.sync.dma_start(out=id32[:], in_=id32_p[:])
            id16 = consts.tile([P, P], f16)
            nc.sync.dma_start(out=id16[:], in_=id16_p[:])
            gam_t = consts.tile([P, D_OUT], f32)
            nc.sync.dma_start(out=gam_t[:], in_=gam_p[:])
            bet_t = consts.tile([P, D_OUT], f32)
            nc.sync.dma_start(out=bet_t[:], in_=bet_p[:])
            ones32 = consts.tile([1, P], f32)
            nc.vector.memset(ones32[:], 1.0)
            ones16 = consts.tile([1, P], f16)
            nc.vector.memset(ones16[:], 1.0)
            iota16 = consts.tile([P, P], f16)
            nc.sync.dma_start(out=iota16[:], in_=iota_p[:])
            eps_t = consts.tile([P, 1], f32)
            nc.vector.memset(eps_t[:], EPS)
            zero1 = consts.tile([P, 1], f32)
            nc.vector.memset(zero1[:], 0.0)

            # ---- phase A: node transforms ----
            with tc.tile_pool(name="pa_sb", bufs=3) as pa, \
                 tc.tile_pool(name="pa_ps", bufs=3, space="PSUM") as pap:
                def node_tiles(x_param, n_tiles, w_t, b_t, out_tab):
                    for t in range(n_tiles):
                        xt = pa.tile([P, D_IN], f32, tag="xt")
                        nc.sync.dma_start(out=xt[:], in_=x_param[t * P:(t + 1) * P, :])
                        xT_ps = pap.tile([P, P], f32, tag="xT", space="PSUM")
                        nc.tensor.matmul(out=xT_ps[:], lhsT=xt[:], rhs=id32[:],
                                         start=True, stop=True)
                        xT = pa.tile([P, P], f32, tag="xTs")
                        nc.scalar.copy(out=xT[:], in_=xT_ps[:])
                        o_ps = pap.tile([P, CH], f32, tag="ops", space="PSUM")
                        nc.tensor.matmul(out=o_ps[:], lhsT=xT[:], rhs=w_t[:],
                                         start=True, stop=False)
                        nc.tensor.matmul(out=o_ps[:], lhsT=ones32[:], rhs=b_t[:],
                                         start=False, stop=True)
                        o16 = pa.tile([P, CH], f16, tag="o16")
                        nc.scalar.copy(out=o16[:], in_=o_ps[:])
                        nc.sync.dma_start(out=out_tab[t * P:(t + 1) * P, :], in_=o16[:])
                node_tiles(X_full, NFT, w_l, b_l, xl16)
                node_tiles(X_local, NT, w_r, b_r, xr16)
                z16 = pa.tile([P, CH], f16, tag="z16")
                nc.vector.memset(z16[:], 0.0)
                nc.sync.dma_start(out=xr16[NPAD:NPAD + P, :], in_=z16[:])

            # ---- phase B/C pools ----
            sp_pool = ctx.enter_context(tc.tile_pool(name="spans", bufs=2))
            ix_pool = ctx.enter_context(tc.tile_pool(name="idx", bufs=2))
            ck_pool = ctx.enter_context(tc.tile_pool(name="chunk", bufs=2))
            al_pool = ctx.enter_context(tc.tile_pool(name="alpha", bufs=3))
            agg_ps = ctx.enter_context(tc.tile_pool(name="aggps", bufs=4, space="PSUM"))
            p2_ps = ctx.enter_context(tc.tile_pool(name="p2ps", bufs=2, space="PSUM"))
            p2_sb = ctx.enter_context(tc.tile_pool(name="p2sb", bufs=3))

            psum_of_tile = {}
            qn = [0]

            def next_q():
                q = qn[0] % NQ
                qn[0] += 1
                return q

            def phase_c(t):
                ps = psum_of_tile.pop(t)
                recip = p2_sb.tile([P, H], f32, tag="recip")
                nc.vector.reciprocal(recip[:], ps[:, CH:CH + H])
                attn = p2_sb.tile([P, CH], f16, tag="attn")
                num3 = ps[:, 0:CH].rearrange("p (h c) -> p h c", h=H)
                r3 = bass.AP(tensor=recip.tensor, offset=recip.offset,
                             ap=[recip.ap[0], recip.ap[1], [0, D_OUT]])
                nc.vector.tensor_tensor(
                    out=attn[:].rearrange("p (h c) -> p h c", h=H),
                    in0=num3, in1=r3, op=AT.mult)
                aT_ps = p2_ps.tile([P, P], f32, tag="tr", space="PSUM")
                nc.tensor.matmul(out=aT_ps[:], lhsT=attn[:], rhs=id16[:],
                                 start=True, stop=True)
                aT = p2_sb.tile([P, P], f16, tag="aT")
                nc.scalar.copy(out=aT[:], in_=aT_ps[:])
                h_ps = p2_ps.tile([P, D_OUT], f32, tag="h", space="PSUM")
                nc.tensor.matmul(out=h_ps[:], lhsT=aT[:], rhs=w_f[:],
                                 start=True, stop=False)
                nc.tensor.matmul(out=h_ps[:], lhsT=ones16[:], rhs=bf_e[:],
                                 start=False, stop=True)
                musum = p2_sb.tile([P, 1], f32, tag="musum")
                nc.vector.tensor_reduce(out=musum[:], in_=h_ps[:], op=AT.add,
                                        axis=mybir.AxisListType.X)
                mu = p2_sb.tile([P, 1], f32, tag="mu")
                nc.scalar.mul(mu[:], musum[:], 1.0 / D_OUT)
                hc = p2_sb.tile([P, D_OUT], f32, tag="hc")
                mu_b = bass.AP(tensor=mu.tensor, offset=mu.offset,
                               ap=[mu.ap[0], [0, D_OUT]])
                nc.vector.tensor_tensor(out=hc[:], in0=h_ps[:], in1=mu_b,
                                        op=AT.subtract)
                sq = p2_sb.tile([P, D_OUT], f16, tag="sq")
                vs = p2_sb.tile([P, 1], f32, tag="vs")
                nc.scalar.activation(out=sq[:], in_=hc[:], func=AF.Square,
                                     bias=zero1[:], accum_out=vs[:])
                sr = p2_sb.tile([P, 1], f32, tag="sr")
                nc.scalar.activation(out=sr[:], in_=vs[:], func=AF.Sqrt,
                                     bias=eps_t[:], scale=1.0 / D_OUT)
                rstd = p2_sb.tile([P, 1], f32, tag="rstd")
                nc.vector.reciprocal(rstd[:], sr[:])
                outt = p2_sb.tile([P, D_OUT], f32, tag="outt")
                rstd_b = bass.AP(tensor=rstd.tensor, offset=rstd.offset,
                                 ap=[rstd.ap[0], [0, D_OUT]])
                nc.vector.tensor_tensor(out=outt[:], in0=hc[:], in1=rstd_b,
                                        op=AT.mult)
                if not ln_trivial:
                    nc.vector.tensor_tensor(out=outt[:], in0=outt[:], in1=gam_t[:],
                                            op=AT.mult)
                    nc.vector.tensor_tensor(out=outt[:], in0=outt[:], in1=bet_t[:],
                                            op=AT.add)
                nc.sync.dma_start(out=h_out[t * P:(t + 1) * P, :], in_=outt[:])

            for (t0, t1, b0, blo, bhi) in spans:
                nb = blo + bhi
                xl_sp = sp_pool.tile([P, nb, CH], f16, tag="xl")
                xr_sp = sp_pool.tile([P, nb, CH], f16, tag="xr")
                dstt = ix_pool.tile([P, nb], f16, tag="dst")
                nc.sync.dma_start(out=dstt[:], in_=dst16_p[:, b0:b0 + nb])
                if blo > 0:
                    ilo = ix_pool.tile([P, blo * 8], i16, tag="ilo")
                    nc.sync.dma_start(out=ilo[:], in_=xl_idx[:, b0 * 8:(b0 + blo) * 8])
                    nc.gpsimd.dma_gather(
                        xl_sp[:, 0:blo, :], xl16[0:LO_LIM, :], ilo[:],
                        blo * EB, blo * EB, CH, elem_step=CH,
                        single_packet=False, queue_num=next_q())
                if bhi > 0:
                    ihi = ix_pool.tile([P, bhi * 8], i16, tag="ihi")
                    nc.sync.dma_start(out=ihi[:],
                                      in_=xl_idx[:, (b0 + blo) * 8:(b0 + nb) * 8])
                    nc.gpsimd.dma_gather(
                        xl_sp[:, blo:nb, :], xl16[LO_LIM:XL_ROWS, :], ihi[:],
                        bhi * EB, bhi * EB, CH, elem_step=CH,
                        single_packet=False, queue_num=next_q())
                ixr = ix_pool.tile([P, nb * 8], i16, tag="ixr")
                nc.sync.dma_start(out=ixr[:], in_=xr_idx[:, b0 * 8:(b0 + nb) * 8])
                nc.gpsimd.dma_gather(
                    xr_sp[:, 0:nb, :], xr16[:], ixr[:],
                    nb * EB, nb * EB, CH, elem_step=CH,
                    single_packet=False, queue_num=next_q())

                for cb in range(0, nb, CHUNK):
                    ce = min(cb + CHUNK, nb)
                    B = ce - cb
                    xl_c = xl_sp[:, cb:ce, :]
                    s_t = ck_pool.tile([P, CHUNK, CH], f16, tag="s")
                    nc.vector.tensor_tensor(out=s_t[:, 0:B, :], in0=xl_c,
                                            in1=xr_sp[:, cb:ce, :], op=AT.add)
                    t02 = ck_pool.tile([P, CHUNK, CH], f16, tag="t02")
                    nc.scalar.mul(t02[:, 0:B, :], s_t[:, 0:B, :], NEG_SLOPE)
                    lr_t = ck_pool.tile([P, CHUNK, CH], f16, tag="lr")
                    nc.vector.tensor_tensor(out=lr_t[:, 0:B, :], in0=s_t[:, 0:B, :],
                                            in1=t02[:, 0:B, :], op=AT.max)
                    pr_t = ck_pool.tile([P, CHUNK, CH], f16, tag="pr")
                    att_b = bass.AP(tensor=att_t.tensor, offset=att_t.offset,
                                    ap=[att_t.ap[0], [0, B], att_t.ap[1]])
                    nc.vector.tensor_tensor(out=pr_t[:, 0:B, :], in0=lr_t[:, 0:B, :],
                                            in1=att_b, op=AT.mult)
                    al_t = al_pool.tile([P, CHUNK, H], f32, tag="al")
                    nc.vector.tensor_reduce(
                        out=al_t[:, 0:B, :],
                        in_=pr_t[:, 0:B, :].rearrange("p b (h c) -> p b h c", h=H),
                        op=AT.add, axis=mybir.AxisListType.X)
                    w_t = al_pool.tile([P, CHUNK, H], f32, tag="w")
                    nc.scalar.activation(out=w_t[:, 0:B, :], in_=al_t[:, 0:B, :],
                                         func=AF.Exp, bias=zero1[:])
                    msg = ck_pool.tile([P, CHUNK, CH + 4], f16, tag="msg")
                    wv = w_t[:, 0:B, :]
                    w_b = bass.AP(tensor=w_t.tensor, offset=wv.offset,
                                  ap=[wv.ap[0], wv.ap[1], wv.ap[2], [0, D_OUT]])
                    nc.vector.tensor_tensor(
                        out=msg[:, 0:B, 0:CH].rearrange("p b (h c) -> p b h c", h=H),
                        in0=xl_c.rearrange("p b (h c) -> p b h c", h=H),
                        in1=w_b, op=AT.mult)
                    nc.vector.tensor_copy(out=msg[:, 0:B, CH:CH + H],
                                          in_=w_t[:, 0:B, :])
                    S_t = ck_pool.tile([P, CHUNK, P], f16, tag="S")
                    io_b = bass.AP(tensor=iota16.tensor, offset=iota16.offset,
                                   ap=[iota16.ap[0], [0, B], [1, P]])
                    dv = dstt[:, cb:ce]
                    d_b = bass.AP(tensor=dstt.tensor, offset=dv.offset,
                                  ap=[dv.ap[0], dv.ap[1], [0, P]])
                    nc.vector.tensor_tensor(out=S_t[:, 0:B, :], in0=io_b, in1=d_b,
                                            op=AT.is_equal)
                    for j in range(B):
                        blk = b0 + cb + j
                        t = int(blk_tile[blk])
                        if first_blk[blk]:
                            agg_t = agg_ps.tile([P, CH + H], f32,
                                                tag="agg", space="PSUM")
                            psum_of_tile[t] = agg_t
                        nc.tensor.matmul(out=psum_of_tile[t][:],
                                         lhsT=S_t[:, j, :],
                                         rhs=msg[:, j, 0:CH + H],
                                         start=bool(first_blk[blk]),
                                         stop=bool(last_blk[blk]))
                        if last_blk[blk]:
                            phase_c(t)

    nc.compile()
    return nc


def _make_iota_row():
    # iota along free dim as f16, replicated rows
    return np.tile(np.arange(128, dtype=np.float16)[None, :], (128, 1))


def kernel(X, E, attr, W_l, b_l, W_r, b_r, att, bias, W_f, b_f, gamma, beta):
    X = np.asarray(X, np.float32)
    E = np.asarray(E)
    sched, per_core = _preprocess(E)

    key = (tuple(sched["B_lo"].tolist()), tuple(sched["B_hi"].tolist()))
    ln_trivial = bool(np.all(np.asarray(gamma) == 1.0) and np.all(np.asarray(beta) == 0.0))
    ck = (key, ln_trivial)
    if ck not in _CACHE:
        _CACHE[ck] = _build(sched, ln_trivial)
    nc = _CACHE[ck]

    X_pad = np.zeros((XL_ROWS, D_IN), np.float32)
    X_pad[:N] = X
    att01 = np.concatenate([np.asarray(att[0]), np.asarray(att[1])]).astype(np.float16)
    att_t = np.tile(att01[None, :], (128, 1))
    bf_eff = (np.asarray(b_f, np.float64)
              + np.asarray(bias, np.float64) @ np.asarray(W_f, np.float64)
              ).astype(np.float16)[None, :]
    idm = np.eye(128, dtype=np.float32)
    gam_t = np.tile(np.asarray(gamma, np.float32)[None, :], (128, 1))
    bet_t = np.tile(np.asarray(beta, np.float32)[None, :], (128, 1))

    in_maps = []
    for c in range(NCORES):
        xl_w, xr_w, dst16 = per_core[c]
        X_loc = np.zeros((NPAD, D_IN), np.float32)
        X_loc[:NPC] = X[c * NPC:(c + 1) * NPC]
        in_maps.append(dict(
            X_full=X_pad, X_local=X_loc, xl_idx=xl_w, xr_idx=xr_w, dst16=dst16,
            W_l=np.asarray(W_l, np.float32), W_r=np.asarray(W_r, np.float32),
            b_l=np.asarray(b_l, np.float32)[None, :],
            b_r=np.asarray(b_r, np.float32)[None, :],
            att_t=att_t, W_f16=np.asarray(W_f, np.float16),
            bf_eff=bf_eff, id_f32=idm, id_f16=idm.astype(np.float16),
            iota_t=_make_iota_row(),
            gamma_t=gam_t, beta_t=bet_t,
        ))

    res = run_bass_kernel_spmd(nc, in_maps, list(range(NCORES)))
    h = np.concatenate([res.results[c]["h_out"][:NPC] for c in range(NCORES)], axis=0)
    return (h, np.asarray(E), np.asarray(attr))
